# revision 13
# baseline (speedup 1.0000x reference)
"""Trainium2 Bass kernel for CoordGlobalIUVHeadAfterMaskBranch.

Reference computation (N=4, C=128, H=W=160, M=64, OUT=75, stride=8):
  1. rel_coord[n,2,H,W]: per-image segment-mean over instances of
     ((inst_xy - pixel_loc)/soi[fpn]) * s_logits
  2. x = concat([rel_coord, iuv_feats]) -> 4x (conv3x3 C->128 + relu)
  3. conv1x1 128->75
  4. aligned_bilinear x2 -> [4, 75, 320, 320]

Sharding: 8 cores = 4 images x 2 row-halves. Bottom halves are fed
row-FLIPPED data (and dy-flipped conv taps) so that every core sees the
image boundary at the same tile rows -> one uniform SPMD program; the
per-layer conv zero-padding at the boundary becomes a fixed-row memset.
Conv tower runs as 9 shifted fp16 matmuls per 3x3 layer in a width-padded
(162-pitch) flat layout; rel_coord collapses to a [64->4] instance matmul
plus two vector ops; the x2 aligned-bilinear upsample is separable
(strided copies + averages) spread over ACT/DVE/GPSIMD.

Each core emits output rows j in [0,161) of its (possibly flipped) half;
the host assembles: row0/col0 replication quirks of aligned_bilinear are
restored by copying row1->row0 / col1->col0 of the full output.
"""

import os

import numpy as np

import concourse.bass as bass
import concourse.mybir as mybir
from concourse.bass_utils import run_bass_kernel_spmd
from concourse.tile import TileContext

F16 = mybir.dt.float16
F32 = mybir.dt.float32

# Problem constants (hardcoded per task contract)
N_IMG, C, H, W = 4, 128, 160, 160
M_INST, C_OUT, STRIDE = 64, 75, 8
N_CORES = 8

HALF = 80              # conv rows of output owned by each core
WPAD = W + 2           # 162: row pitch with 2 zero pad cols
R_IN = 90              # data rows: local g in [-5, 85)
TILE_ROWS = R_IN + 2   # 92: + top/bottom zero pad rows
FLAT = TILE_ROWS * WPAD          # 14904 flat elements per partition
SPAN_IN = R_IN * WPAD            # 14580: instance-matmul span (tile rows 1..91)
IN_BASE = 1 * WPAD               # flat offset of tile row 1

# Layer L output rows in tile coords (tile row t = local row g + 6;
# local g<0 is outside the image at this core's boundary edge)
LO = [2, 3, 4, 5]
HI = [90, 89, 88, 87]
SPAN0_BASE = LO[0] * WPAD        # 324
SPAN0 = (HI[0] - LO[0]) * WPAD   # 14256 (R18 / conv0-out span)
MMCH = 512                       # matmul free-dim chunk (1 PSUM bank fp32)
UPB = 8                          # upsample block: 8 x-rows
OUTR = 161                       # output rows emitted per core

_PROG_CACHE = {}
_last_results = None  # test.py reads exec_time_ns from here


def _ceil_div(a, b):
    return (a + b - 1) // b


def _split_multiwaits(nc):
    """Walrus codegen allows only one sync-wait slot per instruction; hoist
    extra waits into standalone same-engine EventSemaphore instructions."""
    for func in nc.m.functions:
        for blk in func.blocks:
            insts = blk.instructions
            out = []
            for inst in insts:
                si = inst.sync_info
                if si is not None and len(si.on_wait) > 1:
                    waits = list(si.on_wait)
                    for k, w in enumerate(waits[:-1]):
                        out.append(mybir.InstEventSemaphore(
                            name=f"{inst.name}-ws{k}",
                            ins=[], outs=[],
                            sync_info=mybir.SyncInfo(on_wait=[w], on_update=[]),
                            engine=inst.engine,
                        ))
                    inst.sync_info = mybir.SyncInfo(
                        on_wait=[waits[-1]], on_update=list(si.on_update))
                out.append(inst)
            blk.instructions = out
    return nc


def _build_program(reps=1):
    nc = bass.Bass()

    feats_d = nc.declare_dram_parameter("feats", [C, FLAT], F16, isOutput=False)
    slog_d = nc.declare_dram_parameter("slog", [M_INST, FLAT], F16, isOutput=False)
    locs_d = nc.declare_dram_parameter("locs", [2, SPAN_IN], F16, isOutput=False)
    coef_d = nc.declare_dram_parameter("coef", [M_INST, 4], F16, isOutput=False)
    # wblob[ci, idx, co]: idx 0-8 L0 feat taps (dy*3+dx), 9-17 L1, 18-26 L2,
    # 27-35 L3, 36: rows 0:18 = w0 rel taps (c*9+dy*3+dx), 37: cols 0:75 = wf
    wblob_d = nc.declare_dram_parameter("wblob", [C, 38 * C], F16, isOutput=False)
    bias_d = nc.declare_dram_parameter("bias", [C, 5], F32, isOutput=False)
    out_d = nc.declare_dram_parameter("out", [C_OUT, OUTR, 2 * W], F32, isOutput=True)

    with TileContext(nc) as tc:
        with (
            tc.tile_pool(name="const", bufs=1) as const_pool,
            tc.tile_pool(name="ab", bufs=1) as ab_pool,
            tc.tile_pool(name="psum_conv", bufs=4, space="PSUM") as pconv,
        ):
            wtile = const_pool.tile([C, 38, C], F16)
            nc.sync.dma_start(out=wtile[:], in_=wblob_d[:].rearrange("p (i k) -> p i k", k=C))
            btile = const_pool.tile([C, 5], F32)
            nc.sync.dma_start(out=btile[:], in_=bias_d[:])
            coef_t = const_pool.tile([M_INST, 4], F16)
            nc.sync.dma_start(out=coef_t[:], in_=coef_d[:])

            bufA = ab_pool.tile([C, FLAT], F16)
            bufB = ab_pool.tile([C, FLAT], F16)

            def conv_layer(layer, src, dst, tap0, r18=None):
                lo, hi = LO[layer], HI[layer]
                base = lo * WPAD
                span = (hi - lo) * WPAD
                for cidx in range(_ceil_div(span, MMCH)):
                    q0 = base + MMCH * cidx
                    cs = min(MMCH, base + span - q0)
                    ps = pconv.tile([C, MMCH], F32, tag="convps")
                    for t in range(9):
                        dy, dx = t // 3, t % 3
                        off = q0 + (dy - 1) * WPAD + (dx - 1)
                        nc.tensor.matmul(
                            ps[:, :cs],
                            wtile[:, tap0 + t, :],
                            src[:, off:off + cs],
                            start=(t == 0),
                            stop=(t == 8 and r18 is None),
                        )
                    if r18 is not None:
                        nc.tensor.matmul(
                            ps[:, :cs],
                            wtile[:18, 36, :],
                            r18[:, q0 - SPAN0_BASE:q0 - SPAN0_BASE + cs],
                            start=False,
                            stop=True,
                        )
                    nc.scalar.activation(
                        out=dst[:, q0:q0 + cs],
                        in_=ps[:, :cs],
                        func=mybir.ActivationFunctionType.Relu,
                        bias=btile[:, layer:layer + 1],
                    )
                # rows above the image boundary must be exactly zero for the
                # next layer's padding semantics (tile rows [lo, 6))
                nc.vector.memset(dst[:, lo * WPAD:6 * WPAD], 0.0)
                # zero the pad cols of rows [6, hi) so they act as x-padding
                pad_ap = bass.AP(
                    tensor=dst.tensor,
                    offset=dst.offset + 6 * WPAD + W,
                    ap=[dst.ap[0], [WPAD, hi - 6], [1, 2]],
                )
                nc.vector.memset(pad_ap, 0.0)

            import contextlib
            loop_cm = tc.For_i(0, reps, 1) if reps > 1 else contextlib.nullcontext()
            with loop_cm:
                emit_body_marker = True  # noqa: F841
                nc.any.memset(bufA[:], 0.0)
                nc.any.memset(bufB[:], 0.0)
                _emit_phases(nc, tc, conv_layer, bufA, bufB, wtile, btile, coef_t,
                             feats_d, slog_d, locs_d, out_d)

    return _split_multiwaits(nc)


def _emit_phases(nc, tc, conv_layer, bufA, bufB, wtile, btile, coef_t,
                 feats_d, slog_d, locs_d, out_d):
    if True:
        if True:
            # ---------------- phase 1: rel_coord + conv0 ----------------
            with (
                tc.tile_pool(name="ph1", bufs=1) as ph1,
                tc.tile_pool(name="ph1small", bufs=3) as ph1s,
                tc.tile_pool(name="psum_inst", bufs=2, space="PSUM") as pinst,
            ):
                feats = ph1.tile([C, FLAT], F16)
                third = (TILE_ROWS + 2) // 3 * WPAD
                for i in range(3):
                    a = i * third
                    b = min(FLAT, a + third)
                    nc.sync.dma_start(out=feats[:, a:b], in_=feats_d[:, a:b])

                locs = ph1.tile([2, SPAN_IN], F16)
                nc.sync.dma_start(out=locs[:], in_=locs_d[:])

                rel = ph1.tile([2, FLAT], F16)
                nc.vector.memset(rel[:, 0:WPAD], 0.0)                  # top pad row
                nc.vector.memset(rel[:, FLAT - WPAD:FLAT], 0.0)        # bottom pad row

                # instance matmul: S = coef.T @ s_logits chunks, then
                # rel = S[0:2] - locs * S[2:4]
                ninst = _ceil_div(SPAN_IN, MMCH)
                for cidx in range(ninst):
                    q0 = IN_BASE + MMCH * cidx
                    cs = min(MMCH, IN_BASE + SPAN_IN - q0)
                    sc = ph1s.tile([M_INST, MMCH], F16, tag="slogc")
                    nc.sync.dma_start(out=sc[:, :cs], in_=slog_d[:, q0:q0 + cs])
                    ps12 = pinst.tile([2, MMCH], F32, tag="instps12")
                    ps34 = pinst.tile([2, MMCH], F32, tag="instps34")
                    nc.tensor.matmul(ps12[:, :cs], coef_t[:, 0:2], sc[:, :cs],
                                     start=True, stop=True)
                    nc.tensor.matmul(ps34[:, :cs], coef_t[:, 2:4], sc[:, :cs],
                                     start=True, stop=True)
                    tmp = ph1s.tile([2, MMCH], F32, tag="reltmp")
                    nc.vector.tensor_mul(
                        tmp[:, :cs],
                        locs[:, q0 - IN_BASE:q0 - IN_BASE + cs],
                        ps34[:, :cs],
                    )
                    nc.vector.tensor_sub(rel[:, q0:q0 + cs], ps12[:, :cs], tmp[:, :cs])

                # R18: 18 shifted copies of rel (c,dy,dx) over the conv0 out span
                r18 = ph1.tile([18, SPAN0], F16)
                qsz = _ceil_div(SPAN0, 4)
                for p in range(4):
                    qa = p * qsz
                    qb = min(SPAN0, qa + qsz)
                    for c in range(2):
                        for dy in range(3):
                            src = bass.AP(
                                tensor=rel.tensor,
                                offset=rel.offset + SPAN0_BASE + qa
                                + (dy - 1) * WPAD - 1 + c * rel.ap[0][0],
                                ap=[[rel.ap[0][0], 1], [1, 3], [1, qb - qa]],
                            )
                            nc.sync.dma_start(
                                out=r18[c * 9 + dy * 3:c * 9 + dy * 3 + 3, qa:qb],
                                in_=src)

                conv_layer(0, feats, bufA, 0, r18=r18)

            conv_layer(1, bufA, bufB, 9)
            conv_layer(2, bufB, bufA, 18)
            conv_layer(3, bufA, bufB, 27)

            # ---------------- phase 3: 1x1 conv + aligned bilinear x2 --------
            # x rows: tile rows t in [5, 87) (local g in [-1, 81)).
            # out row j (0..160): odd j=2i+1 <- x[i] (t=i+6);
            # even j=2i+2 <- avg(x[i], x[i+1]) pairs (t, t+1), t = j/2+5.
            with (
                tc.tile_pool(name="up1", bufs=2) as up1,
                tc.tile_pool(name="up2", bufs=2) as up2,
                tc.tile_pool(name="psum_f", bufs=2, space="PSUM") as pfin,
            ):
                nblk = _ceil_div(87 - 5, UPB)
                for blk in range(nblk):
                    tb0 = 5 + UPB * blk
                    tb1 = min(tb0 + UPB, 87)
                    tb1x = min(tb1 + 1, 87)
                    nr = tb1x - tb0          # 9 (blk<9), 2 (blk=9)
                    base = tb0 * WPAD
                    span = nr * WPAD

                    x_blk = up1.tile([C_OUT, (UPB + 1) * WPAD], F32, tag="x")
                    xh_blk = up1.tile([C_OUT, (UPB + 1) * WPAD], F32, tag="xh")
                    xq_blk = up1.tile([C_OUT, (UPB + 1) * WPAD], F32, tag="xq")
                    for cidx in range(_ceil_div(span, MMCH)):
                        q0 = base + MMCH * cidx
                        cs = min(MMCH, base + span - q0)
                        ps = pfin.tile([C_OUT, MMCH], F32, tag="finps")
                        nc.tensor.matmul(
                            ps[:, :cs], wtile[:, 37, :C_OUT], bufB[:, q0:q0 + cs],
                            start=True, stop=True,
                        )
                        o = q0 - base
                        nc.scalar.activation(
                            out=x_blk[:, o:o + cs], in_=ps[:, :cs],
                            func=mybir.ActivationFunctionType.Copy)
                        nc.scalar.activation(
                            out=xh_blk[:, o:o + cs], in_=ps[:, :cs],
                            func=mybir.ActivationFunctionType.Copy, scale=0.5)
                        nc.scalar.activation(
                            out=xq_blk[:, o:o + cs], in_=ps[:, :cs],
                            func=mybir.ActivationFunctionType.Copy, scale=0.25)

                    xv = x_blk[:].rearrange("p (r w) -> p r w", w=WPAD)
                    xhv = xh_blk[:].rearrange("p (r w) -> p r w", w=WPAD)
                    xqv = xq_blk[:].rearrange("p (r w) -> p r w", w=WPAD)

                    xup = up2.tile([C_OUT, UPB + 1, 2 * W], F32, tag="xup")
                    xuph = up2.tile([C_OUT, UPB + 1, 2 * W], F32, tag="xuph")
                    # odd cols (2k+1) = copy x[k]; even cols (2k+2) = xh[k]+xh[k+1]
                    nc.scalar.activation(
                        out=xup[:, :nr, 1::2], in_=xv[:, :nr, 0:W],
                        func=mybir.ActivationFunctionType.Copy)
                    nc.vector.tensor_add(
                        xup[:, :nr, 2::2], xhv[:, :nr, 0:W - 1], xhv[:, :nr, 1:W])
                    nc.gpsimd.tensor_copy(out=xuph[:, :nr, 1::2], in_=xhv[:, :nr, 0:W])
                    nc.vector.tensor_add(
                        xuph[:, :nr, 2::2], xqv[:, :nr, 0:W - 1], xqv[:, :nr, 1:W])

                    # even output rows: pairs (t, t+1), t in [tb0, min(tb1,86))
                    n_even = min(tb1, 86) - tb0
                    even_blk = up2.tile([C_OUT, UPB, 2 * W], F32, tag="even")
                    nc.vector.tensor_add(
                        even_blk[:, :n_even, :],
                        xuph[:, 0:n_even, :],
                        xuph[:, 1:n_even + 1, :],
                    )
                    j_even0 = 2 * (tb0 - 5)
                    dst = bass.AP(
                        tensor=out_d[:].tensor,
                        offset=j_even0 * 2 * W,
                        ap=[[OUTR * 2 * W, C_OUT], [2 * 2 * W, n_even], [1, 2 * W]],
                    )
                    nc.sync.dma_start(out=dst, in_=even_blk[:, :n_even, :])

                    # odd output rows: t in [max(tb0,6), min(tb1,86))
                    t_odd0 = max(tb0, 6)
                    n_odd = min(tb1, 86) - t_odd0
                    j_odd0 = 2 * (t_odd0 - 6) + 1
                    dst = bass.AP(
                        tensor=out_d[:].tensor,
                        offset=j_odd0 * 2 * W,
                        ap=[[OUTR * 2 * W, C_OUT], [2 * 2 * W, n_odd], [1, 2 * W]],
                    )
                    nc.sync.dma_start(
                        out=dst, in_=xup[:, t_odd0 - tb0:t_odd0 - tb0 + n_odd, :])


def _prep_weights(w0, w1, w2, w3, wf, b0, b1, b2, b3, bf):
    """Returns (wblob_normal, wblob_flipped, bias)."""
    w0, w1, w2, w3, wf = [np.asarray(x, dtype=np.float32) for x in (w0, w1, w2, w3, wf)]
    blobs = []
    for flip in (False, True):
        wblob = np.zeros((C, 38, C), np.float16)
        for li, wl in enumerate([w0, w1, w2, w3]):
            ci0 = 2 if li == 0 else 0
            for t in range(9):
                dy, dx = t // 3, t % 3
                sdy = 2 - dy if flip else dy
                wblob[:, 9 * li + t, :] = wl[:, ci0:ci0 + C, sdy, dx].T.astype(np.float16)
        for c in range(2):
            for t in range(9):
                dy, dx = t // 3, t % 3
                sdy = 2 - dy if flip else dy
                wblob[c * 9 + t, 36, :] = w0[:, c, sdy, dx].astype(np.float16)
        wblob[:, 37, :C_OUT] = wf[:, :, 0, 0].T.astype(np.float16)
        blobs.append(wblob.reshape(C, 38 * C))

    bias = np.zeros((C, 5), np.float32)
    for li, bl in enumerate([b0, b1, b2, b3]):
        bias[:, li] = np.asarray(bl)
    bias[:C_OUT, 4] = np.asarray(bf)
    # the final-conv bias is folded as 0.0 in the ACT Copy; assert it really is 0
    assert np.abs(np.asarray(bf)).max() == 0.0, "nonzero bf not supported"
    return blobs[0], blobs[1], bias


def _prep_inputs(s_logits, iuv_feats, im_inds, instance_locations, fpn_levels,
                 soi, wblob_n, wblob_f, bias):
    """Host-side sharding: per-core prepadded fp16 slices.

    Core 2n+h handles image n, half h. h=1 cores get row-flipped data:
    local row g corresponds to global row (g + 0) for h=0 and (159 - g)
    for h=1; in both cases local g in [0,80) is the owned half and the
    image boundary sits at local g=0.
    """
    im_inds = np.asarray(im_inds).astype(np.int64)
    fpn_levels = np.asarray(fpn_levels).astype(np.int64)
    soi = np.asarray(soi, dtype=np.float64)
    inst = np.asarray(instance_locations, dtype=np.float64)

    counts = np.bincount(im_inds, minlength=N_IMG).astype(np.float64)
    d = soi[fpn_levels]
    gamma = 1.0 / (d * np.maximum(counts[im_inds], 1.0))

    feats16 = np.asarray(iuv_feats).astype(np.float16)   # [N, C, H, W]
    slog16 = np.asarray(s_logits)[:, 0].astype(np.float16)  # [M, H, W]

    in_maps = []
    for core in range(N_CORES):
        n, h = core // 2, core % 2
        feats = np.zeros((C, TILE_ROWS, WPAD), np.float16)
        slog = np.zeros((M_INST, TILE_ROWS, WPAD), np.float16)
        # tile row t holds local row g = t - 6; data rows are local [-5, 85)
        # clipped to the image: local g valid in [0, 85) always (g<0 is
        # outside the image at this core's boundary edge; g in [80,85) is
        # interior halo).
        if h == 0:
            # local g = global g
            feats[:, 6:91, :W] = feats16[n, :, 0:85, :]
            slog[:, 6:91, :W] = slog16[:, 0:85, :]
            gl_rows = np.arange(-5, 85)  # global row of tile rows 1..91
        else:
            # local g = 159 - global
            feats[:, 6:91, :W] = feats16[n, :, 159:74:-1, :]
            slog[:, 6:91, :W] = slog16[:, 159:74:-1, :]
            gl_rows = 159 - np.arange(-5, 85)

        locs = np.zeros((2, R_IN, WPAD), np.float16)
        locs[0, :, :W] = (np.arange(W) * STRIDE + STRIDE // 2)[None, :]
        locs[1, :, :] = (gl_rows * STRIDE + STRIDE // 2)[:, None]

        coef = np.zeros((M_INST, 4), np.float64)
        mine = im_inds == n
        coef[mine, 0] = inst[mine, 0] * gamma[mine]
        coef[mine, 1] = inst[mine, 1] * gamma[mine]
        coef[mine, 2] = gamma[mine]
        coef[mine, 3] = gamma[mine]

        in_maps.append({
            "feats": feats.reshape(C, FLAT),
            "slog": slog.reshape(M_INST, FLAT),
            "locs": locs.reshape(2, SPAN_IN).astype(np.float16),
            "coef": coef.astype(np.float16),
            "wblob": wblob_f if h else wblob_n,
            "bias": bias,
        })
    return in_maps


def kernel(s_logits, iuv_feats, im_inds, instance_locations, fpn_levels,
           iuv_feat_stride, soi, w0, b0, w1, b1, w2, b2, w3, b3, wf, bf):
    global _last_results
    assert int(iuv_feat_stride) == STRIDE

    reps = int(os.environ.get("KERNEL_BENCH_REPS", "1"))
    key = ("prog", reps)
    if key not in _PROG_CACHE:
        _PROG_CACHE[key] = _build_program(reps)
    nc = _PROG_CACHE[key]

    wblob_n, wblob_f, bias = _prep_weights(w0, w1, w2, w3, wf, b0, b1, b2, b3, bf)
    in_maps = _prep_inputs(s_logits, iuv_feats, im_inds, instance_locations,
                           fpn_levels, soi, wblob_n, wblob_f, bias)

    res = run_bass_kernel_spmd(nc, in_maps, list(range(N_CORES)))
    _last_results = res

    out = np.empty((N_IMG, C_OUT, 2 * H, 2 * W), np.float32)
    for core in range(N_CORES):
        n, h = core // 2, core % 2
        shard = res.results[core]["out"]  # [75, 161, 320]
        if h == 0:
            out[n, :, 0:161, :] = shard
        else:
            # device row j maps to global row 320 - j; j in [1, 160)
            out[n, :, 161:320, :] = shard[:, 159:0:-1, :]
    # aligned_bilinear(x2): row 0 == row 1 and col 0 == col 1
    out[:, :, :, 0] = out[:, :, :, 1]
    out[:, :, 0, :] = out[:, :, 1, :]
    return out


# revision 21
# speedup vs baseline: 1.1913x; 1.1913x over previous
"""Trainium2 Bass kernel for CoordGlobalIUVHeadAfterMaskBranch.

Reference computation (N=4, C=128, H=W=160, M=64, OUT=75, stride=8):
  1. rel_coord[n,2,H,W]: per-image segment-mean over instances of
     ((inst_xy - pixel_loc)/soi[fpn]) * s_logits
  2. x = concat([rel_coord, iuv_feats]) -> 4x (conv3x3 C->128 + relu)
  3. conv1x1 128->75
  4. aligned_bilinear x2 -> [4, 75, 320, 320]

Sharding: 8 cores = 4 images x 2 row-halves. Bottom halves are fed
row-FLIPPED data (and dy-flipped conv taps) so that every core sees the
image boundary at the same tile rows -> one uniform SPMD program; the
per-layer conv zero-padding at the boundary becomes a fixed-row memset.
Conv tower runs as 9 shifted fp16 matmuls per 3x3 layer in a width-padded
(162-pitch) flat layout; rel_coord collapses to a [64->4] instance matmul
plus two vector ops; the x2 aligned-bilinear upsample is separable
(strided copies + averages) spread over ACT/DVE/GPSIMD.

Each core emits output rows j in [0,161) of its (possibly flipped) half;
the host assembles: row0/col0 replication quirks of aligned_bilinear are
restored by copying row1->row0 / col1->col0 of the full output.
"""

import os

import numpy as np

import concourse.bass as bass
import concourse.mybir as mybir
from concourse.bass_utils import run_bass_kernel_spmd
from concourse.tile import TileContext

F16 = mybir.dt.float16
F32 = mybir.dt.float32

# Problem constants (hardcoded per task contract)
N_IMG, C, H, W = 4, 128, 160, 160
M_INST, C_OUT, STRIDE = 64, 75, 8
N_CORES = 8

HALF = 80              # conv rows of output owned by each core
WPAD = W + 2           # 162: row pitch with 2 zero pad cols
R_IN = 90              # data rows: local g in [-5, 85)
TILE_ROWS = R_IN + 2   # 92: + top/bottom zero pad rows
FLAT = TILE_ROWS * WPAD          # 14904 flat elements per partition
SPAN_IN = R_IN * WPAD            # 14580: instance-matmul span (tile rows 1..91)
IN_BASE = 1 * WPAD               # flat offset of tile row 1

# Layer L output rows in tile coords (tile row t = local row g + 6;
# local g<0 is outside the image at this core's boundary edge)
LO = [2, 3, 4, 5]
HI = [90, 89, 88, 87]
SPAN0_BASE = LO[0] * WPAD        # 324
SPAN0 = (HI[0] - LO[0]) * WPAD   # 14256 (R18 / conv0-out span)
MMCH = 512                       # matmul free-dim chunk (1 PSUM bank fp32)
UPB = 6                          # upsample block: 6 x-rows
OUTR = 161                       # output rows emitted per core

_PROG_CACHE = {}
_last_results = None  # test.py reads exec_time_ns from here


def _ceil_div(a, b):
    return (a + b - 1) // b


def _split_multiwaits(nc):
    """Walrus codegen allows only one sync-wait slot per instruction; hoist
    extra waits into standalone same-engine EventSemaphore instructions."""
    for func in nc.m.functions:
        for blk in func.blocks:
            insts = blk.instructions
            out = []
            for inst in insts:
                si = inst.sync_info
                if si is not None and len(si.on_wait) > 1:
                    waits = list(si.on_wait)
                    for k, w in enumerate(waits[:-1]):
                        out.append(mybir.InstEventSemaphore(
                            name=f"{inst.name}-ws{k}",
                            ins=[], outs=[],
                            sync_info=mybir.SyncInfo(on_wait=[w], on_update=[]),
                            engine=inst.engine,
                        ))
                    inst.sync_info = mybir.SyncInfo(
                        on_wait=[waits[-1]], on_update=list(si.on_update))
                out.append(inst)
            blk.instructions = out
    return nc


def _build_program(reps=1):
    nc = bass.Bass()

    feats_d = nc.declare_dram_parameter("feats", [C, FLAT], F16, isOutput=False)
    slog_d = nc.declare_dram_parameter("slog", [M_INST, FLAT], F16, isOutput=False)
    locs_d = nc.declare_dram_parameter("locs", [2, SPAN_IN], F16, isOutput=False)
    coef_d = nc.declare_dram_parameter("coef", [M_INST, 4], F16, isOutput=False)
    # wblob[ci, idx, co]: idx 0-8 L0 feat taps (dy*3+dx), 9-17 L1, 18-26 L2,
    # 27-35 L3, 36: rows 0:18 = w0 rel taps (c*9+dy*3+dx), 37: cols 0:75 = wf
    wblob_d = nc.declare_dram_parameter("wblob", [C, 38 * C], F16, isOutput=False)
    bias_d = nc.declare_dram_parameter("bias", [C, 5], F32, isOutput=False)
    out_d = nc.declare_dram_parameter("out", [C_OUT, OUTR, 2 * W], F32, isOutput=True)

    with TileContext(nc) as tc:
        with (
            tc.tile_pool(name="const", bufs=1) as const_pool,
            tc.tile_pool(name="ab", bufs=1) as ab_pool,
            tc.tile_pool(name="psum_conv", bufs=4, space="PSUM") as pconv,
        ):
            wtile = const_pool.tile([C, 38, C], F16)
            nc.sync.dma_start(out=wtile[:], in_=wblob_d[:].rearrange("p (i k) -> p i k", k=C))
            btile = const_pool.tile([C, 5], F32)
            nc.sync.dma_start(out=btile[:], in_=bias_d[:])
            coef_t = const_pool.tile([M_INST, 4], F16)
            nc.sync.dma_start(out=coef_t[:], in_=coef_d[:])

            bufA = ab_pool.tile([C, FLAT], F16)
            bufB = ab_pool.tile([C, FLAT], F16)

            def conv_chunk(layer, src, dst3, tap0, r0c, nr, r18=None):
                # Row-aligned chunk (486 <= 512 fp32 PSUM bank). The ACT relu
                # writes only the real cols [0:W) of each row, so the pad cols
                # of dst stay zero from the initial memset and keep serving as
                # conv x-padding -- no pad fixups, and layers pipeline
                # chunk-wise with no full-layer barrier.
                q0 = r0c * WPAD
                cs = nr * WPAD
                ps = pconv.tile([C, 3 * WPAD], F32, tag="convps")
                for t in range(9):
                    dy, dx = t // 3, t % 3
                    off = q0 + (dy - 1) * WPAD + (dx - 1)
                    nc.tensor.matmul(
                        ps[:, :cs],
                        wtile[:, tap0 + t, :],
                        src[:, off:off + cs],
                        start=(t == 0),
                        stop=(t == 8 and r18 is None),
                    )
                if r18 is not None:
                    nc.tensor.matmul(
                        ps[:, :cs],
                        wtile[:18, 36, :],
                        r18[:, q0 - SPAN0_BASE:q0 - SPAN0_BASE + cs],
                        start=False,
                        stop=True,
                    )
                ps3 = ps[:].rearrange("p (r w) -> p r w", w=WPAD)
                nc.scalar.activation(
                    out=dst3[:, r0c:r0c + nr, 0:W],
                    in_=ps3[:, :nr, 0:W],
                    func=mybir.ActivationFunctionType.Relu,
                    bias=btile[:, layer:layer + 1],
                )

            def conv_layer(layer, src, dst, tap0, r18=None):
                lo, hi = LO[layer], HI[layer]
                dst3 = dst[:].rearrange("p (r w) -> p r w", w=WPAD)
                for r0c in range(lo, hi, 3):
                    conv_chunk(layer, src, dst3, tap0, r0c, min(3, hi - r0c), r18)
                # rows above the image boundary must be exactly zero for the
                # next layer's padding semantics (tile rows [lo, 6))
                nc.vector.memset(dst[:, lo * WPAD:6 * WPAD], 0.0)

            import contextlib
            loop_cm = tc.For_i(0, reps, 1) if reps > 1 else contextlib.nullcontext()
            with loop_cm:
                emit_body_marker = True  # noqa: F841
                nc.any.memset(bufA[:], 0.0)
                nc.any.memset(bufB[:], 0.0)
                _emit_phases(nc, tc, conv_layer, conv_chunk, bufA, bufB, wtile,
                             btile, coef_t, feats_d, slog_d, locs_d, out_d)

    return _split_multiwaits(nc)


def _emit_phases(nc, tc, conv_layer, conv_chunk, bufA, bufB, wtile, btile, coef_t,
                 feats_d, slog_d, locs_d, out_d):
    if True:
        if True:
            # ---------------- phase 1: rel_coord + conv0 ----------------
            with (
                tc.tile_pool(name="ph1", bufs=1) as ph1,
                tc.tile_pool(name="ph1small", bufs=3) as ph1s,
                tc.tile_pool(name="psum_inst", bufs=2, space="PSUM") as pinst,
            ):
                feats = ph1.tile([C, FLAT], F16)
                third = (TILE_ROWS + 5) // 6 * WPAD
                for i in range(6):
                    a = i * third
                    b = min(FLAT, a + third)
                    nc.sync.dma_start(out=feats[:, a:b], in_=feats_d[:, a:b])

                locs = ph1.tile([2, SPAN_IN], F16)
                nc.sync.dma_start(out=locs[:], in_=locs_d[:])

                rel = ph1.tile([2, FLAT], F16)
                nc.vector.memset(rel[:, 0:WPAD], 0.0)                  # top pad row
                nc.vector.memset(rel[:, FLAT - WPAD:FLAT], 0.0)        # bottom pad row

                # instance matmul: S = coef.T @ s_logits chunks, then
                # rel = S[0:2] - locs * S[2:4]
                ninst = _ceil_div(SPAN_IN, MMCH)
                for cidx in range(ninst):
                    q0 = IN_BASE + MMCH * cidx
                    cs = min(MMCH, IN_BASE + SPAN_IN - q0)
                    sc = ph1s.tile([M_INST, MMCH], F16, tag="slogc")
                    nc.sync.dma_start(out=sc[:, :cs], in_=slog_d[:, q0:q0 + cs])
                    ps12 = pinst.tile([2, MMCH], F32, tag="instps12")
                    ps34 = pinst.tile([2, MMCH], F32, tag="instps34")
                    nc.tensor.matmul(ps12[:, :cs], coef_t[:, 0:2], sc[:, :cs],
                                     start=True, stop=True)
                    nc.tensor.matmul(ps34[:, :cs], coef_t[:, 2:4], sc[:, :cs],
                                     start=True, stop=True)
                    tmp = ph1s.tile([2, MMCH], F32, tag="reltmp")
                    nc.vector.tensor_mul(
                        tmp[:, :cs],
                        locs[:, q0 - IN_BASE:q0 - IN_BASE + cs],
                        ps34[:, :cs],
                    )
                    nc.vector.tensor_sub(rel[:, q0:q0 + cs], ps12[:, :cs], tmp[:, :cs])

                # R18: 18 shifted copies of rel (c,dy,dx) over the conv0 out span
                r18 = ph1.tile([18, SPAN0], F16)
                qsz = _ceil_div(SPAN0, 4)
                for p in range(4):
                    qa = p * qsz
                    qb = min(SPAN0, qa + qsz)
                    for c in range(2):
                        for dy in range(3):
                            src = bass.AP(
                                tensor=rel.tensor,
                                offset=rel.offset + SPAN0_BASE + qa
                                + (dy - 1) * WPAD - 1 + c * rel.ap[0][0],
                                ap=[[rel.ap[0][0], 1], [1, 3], [1, qb - qa]],
                            )
                            nc.sync.dma_start(
                                out=r18[c * 9 + dy * 3:c * 9 + dy * 3 + 3, qa:qb],
                                in_=src)

                conv_layer(0, feats, bufA, 0, r18=r18)

            conv_layer(1, bufA, bufB, 9)
            conv_layer(2, bufB, bufA, 18)

            # ------- phase 3: L3 conv + 1x1 conv + aligned bilinear x2 -------
            # L3 chunks are emitted interleaved with the upsample blocks so
            # the upsample's ACT/DVE/Pool/DMA chains overlap L3's matmuls.
            # x rows: tile rows t in [5, 87) (local g in [-1, 81)).
            # out row j (0..160): odd j=2i+1 <- x[i] (t=i+6);
            # even j=2i+2 <- avg(x[i], x[i+1]) pairs (t, t+1), t = j/2+5.
            lo3, hi3 = LO[3], HI[3]
            bufB3 = bufB[:].rearrange("p (r w) -> p r w", w=WPAD)
            l3_state = {"next": lo3, "memset_done": False}

            def emit_l3_upto(row):
                while l3_state["next"] < min(row, hi3):
                    r0c = l3_state["next"]
                    nr3 = min(3, hi3 - r0c)
                    conv_chunk(3, bufA, bufB3, 27, r0c, nr3)
                    l3_state["next"] = r0c + nr3
                if not l3_state["memset_done"]:
                    nc.vector.memset(bufB[:, lo3 * WPAD:6 * WPAD], 0.0)
                    l3_state["memset_done"] = True

            edges = list(range(5, 77, UPB)) + list(range(77, 87, 2)) + [87]
            with (
                tc.tile_pool(name="up1", bufs=4) as up1,
                tc.tile_pool(name="up2", bufs=3) as up2,
                tc.tile_pool(name="psum_f", bufs=2, space="PSUM") as pfin,
            ):
                for tb0, tb1 in zip(edges[:-1], edges[1:]):
                    tb1x = min(tb1 + 1, 87)
                    nr = tb1x - tb0          # x rows this block
                    base = tb0 * WPAD
                    span = nr * WPAD
                    emit_l3_upto(tb1x)

                    x_blk = up1.tile([C_OUT, (UPB + 1) * WPAD], F32, tag="x")
                    for r0c in range(tb0, tb1x, 3):
                        nr3 = min(3, tb1x - r0c)
                        q0 = r0c * WPAD
                        cs = nr3 * WPAD
                        ps = pfin.tile([C_OUT, 3 * WPAD], F32, tag="finps")
                        nc.tensor.matmul(
                            ps[:, :cs], wtile[:, 37, :C_OUT], bufB[:, q0:q0 + cs],
                            start=True, stop=True,
                        )
                        o = q0 - base
                        nc.scalar.activation(
                            out=x_blk[:, o:o + cs], in_=ps[:, :cs],
                            func=mybir.ActivationFunctionType.Copy)

                    xv = x_blk[:].rearrange("p (r w) -> p r w", w=WPAD)
                    xup = up2.tile([C_OUT, UPB + 1, 2 * W], F32, tag="xup")
                    t2 = up1.tile([C_OUT, UPB + 1, W - 1], F32, tag="t2")
                    # odd cols (2k+1) = copy x[k]
                    nc.gpsimd.tensor_copy(out=xup[:, :nr, 1::2], in_=xv[:, :nr, 0:W])
                    # even cols (2k+2) = 0.5*(x[k] + x[k+1])
                    nc.vector.tensor_add(
                        t2[:, :nr, :], xv[:, :nr, 0:W - 1], xv[:, :nr, 1:W])
                    nc.vector.tensor_scalar_mul(
                        xup[:, :nr, 2::2], t2[:, :nr, :], 0.5)

                    # even output rows: 0.5*(xup_t + xup_{t+1}),
                    # pairs (t, t+1) for t in [tb0, min(tb1,86))
                    n_even = min(tb1, 86) - tb0
                    t3 = up2.tile([C_OUT, UPB, 2 * W], F32, tag="t3")
                    even_blk = up2.tile([C_OUT, UPB, 2 * W], F32, tag="even")
                    nc.vector.tensor_add(
                        t3[:, :n_even, :],
                        xup[:, 0:n_even, :],
                        xup[:, 1:n_even + 1, :],
                    )
                    nc.vector.tensor_scalar_mul(
                        even_blk[:, :n_even, :], t3[:, :n_even, :], 0.5)
                    j_even0 = 2 * (tb0 - 5)
                    dst = bass.AP(
                        tensor=out_d[:].tensor,
                        offset=j_even0 * 2 * W,
                        ap=[[OUTR * 2 * W, C_OUT], [2 * 2 * W, n_even], [1, 2 * W]],
                    )
                    nc.sync.dma_start(out=dst, in_=even_blk[:, :n_even, :])

                    # odd output rows: t in [max(tb0,6), min(tb1,86))
                    t_odd0 = max(tb0, 6)
                    n_odd = min(tb1, 86) - t_odd0
                    j_odd0 = 2 * (t_odd0 - 6) + 1
                    dst = bass.AP(
                        tensor=out_d[:].tensor,
                        offset=j_odd0 * 2 * W,
                        ap=[[OUTR * 2 * W, C_OUT], [2 * 2 * W, n_odd], [1, 2 * W]],
                    )
                    nc.sync.dma_start(
                        out=dst, in_=xup[:, t_odd0 - tb0:t_odd0 - tb0 + n_odd, :])


def _prep_weights(w0, w1, w2, w3, wf, b0, b1, b2, b3, bf):
    """Returns (wblob_normal, wblob_flipped, bias)."""
    w0, w1, w2, w3, wf = [np.asarray(x, dtype=np.float32) for x in (w0, w1, w2, w3, wf)]
    blobs = []
    for flip in (False, True):
        wblob = np.zeros((C, 38, C), np.float16)
        for li, wl in enumerate([w0, w1, w2, w3]):
            ci0 = 2 if li == 0 else 0
            for t in range(9):
                dy, dx = t // 3, t % 3
                sdy = 2 - dy if flip else dy
                wblob[:, 9 * li + t, :] = wl[:, ci0:ci0 + C, sdy, dx].T.astype(np.float16)
        for c in range(2):
            for t in range(9):
                dy, dx = t // 3, t % 3
                sdy = 2 - dy if flip else dy
                wblob[c * 9 + t, 36, :] = w0[:, c, sdy, dx].astype(np.float16)
        wblob[:, 37, :C_OUT] = wf[:, :, 0, 0].T.astype(np.float16)
        blobs.append(wblob.reshape(C, 38 * C))

    bias = np.zeros((C, 5), np.float32)
    for li, bl in enumerate([b0, b1, b2, b3]):
        bias[:, li] = np.asarray(bl)
    bias[:C_OUT, 4] = np.asarray(bf)
    # the final-conv bias is folded as 0.0 in the ACT Copy; assert it really is 0
    assert np.abs(np.asarray(bf)).max() == 0.0, "nonzero bf not supported"
    return blobs[0], blobs[1], bias


def _prep_inputs(s_logits, iuv_feats, im_inds, instance_locations, fpn_levels,
                 soi, wblob_n, wblob_f, bias):
    """Host-side sharding: per-core prepadded fp16 slices.

    Core 2n+h handles image n, half h. h=1 cores get row-flipped data:
    local row g corresponds to global row (g + 0) for h=0 and (159 - g)
    for h=1; in both cases local g in [0,80) is the owned half and the
    image boundary sits at local g=0.
    """
    im_inds = np.asarray(im_inds).astype(np.int64)
    fpn_levels = np.asarray(fpn_levels).astype(np.int64)
    soi = np.asarray(soi, dtype=np.float64)
    inst = np.asarray(instance_locations, dtype=np.float64)

    counts = np.bincount(im_inds, minlength=N_IMG).astype(np.float64)
    d = soi[fpn_levels]
    gamma = 1.0 / (d * np.maximum(counts[im_inds], 1.0))

    feats16 = np.asarray(iuv_feats).astype(np.float16)   # [N, C, H, W]
    slog16 = np.asarray(s_logits)[:, 0].astype(np.float16)  # [M, H, W]

    in_maps = []
    for core in range(N_CORES):
        n, h = core // 2, core % 2
        feats = np.zeros((C, TILE_ROWS, WPAD), np.float16)
        slog = np.zeros((M_INST, TILE_ROWS, WPAD), np.float16)
        # tile row t holds local row g = t - 6; data rows are local [-5, 85)
        # clipped to the image: local g valid in [0, 85) always (g<0 is
        # outside the image at this core's boundary edge; g in [80,85) is
        # interior halo).
        if h == 0:
            # local g = global g
            feats[:, 6:91, :W] = feats16[n, :, 0:85, :]
            slog[:, 6:91, :W] = slog16[:, 0:85, :]
            gl_rows = np.arange(-5, 85)  # global row of tile rows 1..91
        else:
            # local g = 159 - global
            feats[:, 6:91, :W] = feats16[n, :, 159:74:-1, :]
            slog[:, 6:91, :W] = slog16[:, 159:74:-1, :]
            gl_rows = 159 - np.arange(-5, 85)

        locs = np.zeros((2, R_IN, WPAD), np.float16)
        locs[0, :, :W] = (np.arange(W) * STRIDE + STRIDE // 2)[None, :]
        locs[1, :, :] = (gl_rows * STRIDE + STRIDE // 2)[:, None]

        coef = np.zeros((M_INST, 4), np.float64)
        mine = im_inds == n
        coef[mine, 0] = inst[mine, 0] * gamma[mine]
        coef[mine, 1] = inst[mine, 1] * gamma[mine]
        coef[mine, 2] = gamma[mine]
        coef[mine, 3] = gamma[mine]

        in_maps.append({
            "feats": feats.reshape(C, FLAT),
            "slog": slog.reshape(M_INST, FLAT),
            "locs": locs.reshape(2, SPAN_IN).astype(np.float16),
            "coef": coef.astype(np.float16),
            "wblob": wblob_f if h else wblob_n,
            "bias": bias,
        })
    return in_maps


def kernel(s_logits, iuv_feats, im_inds, instance_locations, fpn_levels,
           iuv_feat_stride, soi, w0, b0, w1, b1, w2, b2, w3, b3, wf, bf):
    global _last_results
    assert int(iuv_feat_stride) == STRIDE

    reps = int(os.environ.get("KERNEL_BENCH_REPS", "1"))
    key = ("prog", reps)
    if key not in _PROG_CACHE:
        _PROG_CACHE[key] = _build_program(reps)
    nc = _PROG_CACHE[key]

    wblob_n, wblob_f, bias = _prep_weights(w0, w1, w2, w3, wf, b0, b1, b2, b3, bf)
    in_maps = _prep_inputs(s_logits, iuv_feats, im_inds, instance_locations,
                           fpn_levels, soi, wblob_n, wblob_f, bias)

    res = run_bass_kernel_spmd(nc, in_maps, list(range(N_CORES)))
    _last_results = res

    out = np.empty((N_IMG, C_OUT, 2 * H, 2 * W), np.float32)
    for core in range(N_CORES):
        n, h = core // 2, core % 2
        shard = res.results[core]["out"]  # [75, 161, 320]
        if h == 0:
            out[n, :, 0:161, :] = shard
        else:
            # device row j maps to global row 320 - j; j in [1, 160)
            out[n, :, 161:320, :] = shard[:, 159:0:-1, :]
    # aligned_bilinear(x2): row 0 == row 1 and col 0 == col 1
    out[:, :, :, 0] = out[:, :, :, 1]
    out[:, :, 0, :] = out[:, :, 1, :]
    return out
